# revision 25
# baseline (speedup 1.0000x reference)
"""Causal self-attention Trainium2 kernel.

Sharding: 8 cores = (4 batches) x (2 head-groups of 8 heads).
Each core: projections for its 512 channels, causal attention for its 8
heads over its batch, partial out-projection over its 512 channels.
Host: sums the two partials per batch and adds the output bias.

All matmul operands are bf16 (accumulation in fp32 PSUM); softmax,
normalization and the output partials stay fp32.

Key optimizations vs the naive version:
  - x is transposed on the HOST: xb arrives [C, T] so xT loads are plain
    per-fc direct DMAs (no DMA-transpose; clean 4KB-contiguous rows).
  - weight loads split per-fc and spread over 4 engine DMA queues so the
    first projection matmul starts ~10us in, not 44us.
  - v bias comes in pre-broadcast [128, 512] from the host (replaces 16
    PE ones-trick matmuls with the existing DVE copy turned tensor_add).
  - softmax normalize is software-pipelined: per block, reciprocal runs
    via the fast custom-DVE approx, and the PE row-broadcast matmul for
    block b is issued between score groups of block b+1 so the in-order
    PE queue never stalls on the DVE.
  - score groups are issued one group ahead of AV so the PE streams
    scores of group g+1 while ACT exps group g.
  - AV matmuls and stair masking are restricted to the not-always-masked
    query columns of diagonal tiles (exact: the skipped ex columns are
    never read).

Layouts on core (b = fixed batch, channels o in [0,512) local):
  xT   [128f, 8fc, 2048t] bf16 - direct per-fc DMA from host-transposed x
  qT/kT [128o, 4oc, 2048t] bf16 - head h = oc*2+hh on partitions hh*64..+64
  vx   [128t, 16tj, 8h*65] bf16 - v natural + ones column per head (denom)
  scores^T tiles [128j, 512i] f32 psum -> exp on ACT (scale=1/8) -> bf16
  attn^T accum psum [65, 512i] f32: rows 0..63 head out, row 64 denom
  out  psum [128t, 512c] f32 -> sbuf -> DRAM partial
"""

from contextlib import ExitStack

import ml_dtypes
import numpy as np

import concourse.bass as bass
import concourse.mybir as mybir
import concourse.tile as tile

P = 128
C = 1024  # d_model
CL = 512  # local channels (8 heads * 64)
D = 64  # head dim
NH = 8  # local heads
FC = C // P  # 8 f-chunks
OC = CL // P  # 4 o-chunks
F32 = mybir.dt.float32
BF16 = mybir.dt.bfloat16
AF = mybir.ActivationFunctionType
GROUP = 3  # score jt-tiles per exp call (3 psum banks, double buffered)


def _emit(nc, tc, ctx, T):
    NT = T // P  # 128-token chunks
    T4 = T // 512  # 512-token chunks

    xb = nc.dram_tensor("xb", [C, T], BF16, kind="ExternalInput")
    wq_d = nc.dram_tensor("wq", [C, CL], BF16, kind="ExternalInput")
    wk_d = nc.dram_tensor("wk", [C, CL], BF16, kind="ExternalInput")
    wv_d = nc.dram_tensor("wv", [C, CL], BF16, kind="ExternalInput")
    wo_d = nc.dram_tensor("wo", [CL, C], BF16, kind="ExternalInput")
    bq_d = nc.dram_tensor("bq", [CL], F32, kind="ExternalInput")
    bk_d = nc.dram_tensor("bk", [CL], F32, kind="ExternalInput")
    bvb_d = nc.dram_tensor("bvb", [P, CL], BF16, kind="ExternalInput")
    stair_d = nc.dram_tensor("stair", [P, 1024], BF16, kind="ExternalInput")
    outp = nc.dram_tensor("outp", [T, C], F32, kind="ExternalOutput")

    const = ctx.enter_context(tc.tile_pool(name="const", bufs=1))
    ones1 = const.tile([1, P], BF16)
    bq_sb = const.tile([P, OC], F32)
    bk_sb = const.tile([P, OC], F32)
    bvb_sb = const.tile([P, CL], BF16)
    stair_sb = const.tile([P, 1024], BF16)

    qkv = ctx.enter_context(tc.tile_pool(name="qkv", bufs=1))
    qT = qkv.tile([P, OC, T], BF16)
    kT = qkv.tile([P, OC, T], BF16)
    vx = qkv.tile([P, NT, NH * 65], BF16)
    vx5 = vx.rearrange("p n (h u) -> p n h u", u=65)

    # ---------------- projections ----------------
    with (
        tc.tile_pool(name="wpool", bufs=1) as wpool,
        tc.tile_pool(name="xT_pool", bufs=1) as xT_pool,
        tc.tile_pool(name="pj_ps", bufs=8, space="PSUM") as pj_ps,
    ):
        xT = xT_pool.tile([P, FC, T], BF16)
        wq_sb = wpool.tile([P, FC, CL], BF16)
        wk_sb = wpool.tile([P, FC, CL], BF16)
        wv_sb = wpool.tile([P, FC, CL], BF16)
        xbr = xb.rearrange("(fc p) t -> p fc t", p=P)
        wqr = wq_d.rearrange("(fc p) o -> p fc o", p=P)
        wkr = wk_d.rearrange("(fc p) o -> p fc o", p=P)
        wvr = wv_d.rearrange("(fc p) o -> p fc o", p=P)
        # spread the loads over the three DMA-capable queues; per-fc
        # granularity so the first matmuls start as soon as chunk 0 lands.
        # x chunks alternate between two queues to double early supply
        # (the q+k projections consume each xT chunk within ~3us).
        # Constants / memsets issue AFTER these: they are needed late.
        for fc in range(FC):
            if fc % 2 == 0:
                nc.sync.dma_start(xT[:, fc, :], xbr[:, fc, :])
            else:
                nc.gpsimd.dma_start(xT[:, fc, :], xbr[:, fc, :])
            nc.gpsimd.dma_start(wq_sb[:, fc, :], wqr[:, fc, :])
            nc.scalar.dma_start(wk_sb[:, fc, :], wkr[:, fc, :])
        for fc in range(FC):
            nc.sync.dma_start(wv_sb[:, fc, :], wvr[:, fc, :])
        nc.scalar.dma_start(bq_sb[:], bq_d.rearrange("(oc p) -> p oc", p=P))
        nc.scalar.dma_start(bk_sb[:], bk_d.rearrange("(oc p) -> p oc", p=P))
        nc.scalar.dma_start(bvb_sb[:], bvb_d[:])
        nc.scalar.dma_start(stair_sb[:], stair_d[:])
        nc.gpsimd.memset(ones1[:], 1.0)
        nc.gpsimd.memset(vx5[:, :, :, 64:65], 1.0)

        for oc in range(OC):
            # q and k interleaved per fc chunk: 8 matmuls per arriving
            # xT chunk instead of 4, so the PE keeps up with the DMAs.
            pq = [
                pj_ps.tile([P, 512], F32, tag="pj", name=f"pjq{oc}_{tt}")
                for tt in range(T4)
            ]
            pk = [
                pj_ps.tile([P, 512], F32, tag="pj", name=f"pjk{oc}_{tt}")
                for tt in range(T4)
            ]
            for fc in range(FC):
                for w_sb, pss in ((wq_sb, pq), (wk_sb, pk)):
                    for tt in range(T4):
                        nc.tensor.matmul(
                            pss[tt][:],
                            w_sb[:, fc, oc * P : (oc + 1) * P],
                            xT[:, fc, tt * 512 : (tt + 1) * 512],
                            start=(fc == 0),
                            stop=(fc == FC - 1),
                        )
            for b_sb, dT, pss in ((bq_sb, qT, pq), (bk_sb, kT, pk)):
                for tt in range(T4):
                    nc.vector.tensor_scalar_add(
                        dT[:, oc, tt * 512 : (tt + 1) * 512],
                        pss[tt][:],
                        b_sb[:, oc : oc + 1],
                    )
        for s in range(NT):
            ps = pj_ps.tile([P, 512], F32, tag="pj", name=f"pjv{s}")
            for fc in range(FC):
                nc.tensor.matmul(
                    ps[:],
                    xT[:, fc, s * P : (s + 1) * P],
                    wv_sb[:, fc, :],
                    start=(fc == 0),
                    stop=(fc == FC - 1),
                )
            nc.vector.tensor_add(
                vx5[:, s, :, 0:64],
                ps[:].rearrange("p (h d) -> p h d", d=D),
                bvb_sb[:].rearrange("p (h d) -> p h d", d=D),
            )

    # ---------------- attention ----------------
    wo_pool = ctx.enter_context(tc.tile_pool(name="wo_pool", bufs=1))
    attT_pool = ctx.enter_context(tc.tile_pool(name="attT_pool", bufs=1))
    wo_sb = wo_pool.tile([P, OC, C], BF16)
    nc.gpsimd.dma_start(wo_sb[:], wo_d.rearrange("(oc p) c -> p oc c", p=P))
    attT = attT_pool.tile([P, OC, T], BF16)

    with (
        tc.tile_pool(name="exp_pool", bufs=3) as exp_pool,
        tc.tile_pool(name="nrm", bufs=2) as nrm_pool,
        tc.tile_pool(name="sc_ps", bufs=2, space="PSUM") as sc_ps_pool,
        tc.tile_pool(name="at_ps", bufs=2, space="PSUM") as at_ps_pool,
    ):
        pending = None  # (at, rc_bf, oc, base, ic) awaiting normalize
        pending_recips = []  # lazy [1,128] reciprocal chunks

        def issue_recips(k):
            for _ in range(min(k, len(pending_recips))):
                r_rc, r_at, q4 = pending_recips.pop(0)
                with nc.allow_low_precision(reason="softmax recip bcast"):
                    nc.vector.reciprocal(
                        r_rc[:, q4 * 128 : (q4 + 1) * 128],
                        r_at[64:65, q4 * 128 : (q4 + 1) * 128],
                    )

        def flush_normalize():
            nonlocal pending
            if pending is None:
                return
            issue_recips(4)
            p_at, p_rc, p_oc, p_base, p_ic = pending
            pending = None
            # broadcast recip row into psum rows 64..127 (K=1 outer)
            nc.tensor.matmul(
                p_at[64:128, :], ones1[:, 0:64], p_rc[:], start=True, stop=True
            )
            tmp = nrm_pool.tile([64, 512], F32, tag="tmp")
            nc.scalar.copy(tmp[:], p_at[0:64, :])
            nc.vector.tensor_mul(
                attT[p_base : p_base + D, p_oc, p_ic * 512 : (p_ic + 1) * 512],
                tmp[:],
                p_at[64:128, :],
            )

        # flat software pipeline over all (head, query-block) blocks: the
        # PE issue order interleaves score groups one ahead of AV groups
        # ACROSS block boundaries, so exp/stair latency is always covered
        # by score streaming, and the normalize broadcast of block b sits
        # ~2 score groups + 1 AV group after b's last AV (covers the
        # 3.4us DVE reciprocal).
        def issue_av(s):
            nonlocal pending
            (s_at, s_h, s_ic, s_njt, grp, ex, is_last, gi) = s
            if gi == 1:
                flush_normalize()
            for si, jt in enumerate(grp):
                d = jt - s_ic * 4
                lo = d * P if d > 0 else 0
                nc.tensor.matmul(
                    s_at[0:65, lo:512],
                    vx5[:, jt, s_h, :],
                    ex[:, si, lo:512],
                    start=(jt == 0),
                    stop=(jt == s_njt - 1),
                )

        def finish_block(s):
            nonlocal pending
            (s_at, s_h, s_ic, s_njt, grp, ex, is_last, gi) = s
            rc = nrm_pool.tile([1, 512], BF16, tag="rc")
            # reciprocal as 4 lazily-issued [1,128] chunks: a single [1,512]
            # reciprocal is 3.4us of in-order DVE queue time that blocks the
            # next block's stair masks (which gate its AV matmuls); chunks
            # issued between stair groups let the DVE interleave them
            for q4 in range(4):
                pending_recips.append((rc, s_at, q4))
            oc, hh = s_h // 2, s_h % 2
            pending = (s_at, rc, oc, hh * 64, s_ic)

        # Block order pairs a small block with a big one (hh0 ascending ic
        # interleaved with hh1 descending, parity flipped per oc) so the
        # ~3.8us of reciprocal chunks of a just-finished small block always
        # has a big block's scores/AVs to hide behind.
        block_seq = []
        for oc in range(OC):
            ics = list(range(T4))
            a, b = (ics, ics[::-1]) if oc % 2 == 0 else (ics[::-1], ics)
            for i in range(T4):
                block_seq.append((oc, 0, a[i]))
                block_seq.append((oc, 1, b[i]))

        staged = None  # one score-group lookahead
        if True:
            for oc, hh, ic in block_seq:
                h = oc * 2 + hh
                base = hh * 64
                if True:
                    njt = ic * 4 + 4
                    at = at_ps_pool.tile([P, 512], F32)
                    groups = [
                        list(range(g0, min(g0 + GROUP, njt)))
                        for g0 in range(0, njt, GROUP)
                    ]
                    for gi, grp in enumerate(groups):
                        n = len(grp)
                        sc = sc_ps_pool.tile([P, GROUP, 512], F32)
                        for si, jt in enumerate(grp):
                            nc.tensor.matmul(
                                sc[:, si, :],
                                kT[base : base + D, oc, jt * P : (jt + 1) * P],
                                qT[base : base + D, oc, ic * 512 : (ic + 1) * 512],
                                start=True,
                                stop=True,
                            )
                        ex = exp_pool.tile([P, GROUP, 512], BF16)
                        nc.scalar.activation(
                            ex[:, 0:n, :], sc[:, 0:n, :], AF.Exp, scale=0.125
                        )
                        for si, jt in enumerate(grp):
                            d = jt - ic * 4
                            if d >= 0:
                                # zero the sub-diagonal triangle of the
                                # 128-wide diagonal strip; columns left of
                                # it are skipped by the restricted AV.
                                nc.vector.tensor_mul(
                                    ex[:, si, d * P : (d + 1) * P],
                                    ex[:, si, d * P : (d + 1) * P],
                                    stair_sb[:, 512 : 512 + P],
                                )
                        issue_recips(2)
                        if staged is not None:
                            issue_av(staged)
                            if staged[6]:
                                finish_block(staged)
                        staged = (at, h, ic, njt, grp, ex, gi == len(groups) - 1, gi)
        issue_av(staged)
        finish_block(staged)
        flush_normalize()

    # ---------------- out-projection ----------------
    with (
        tc.tile_pool(name="op_ps", bufs=4, space="PSUM") as op_ps,
        tc.tile_pool(name="ob_pool", bufs=4) as ob_pool,
    ):
        for s16 in range(NT):
            for ch in range(2):
                ps = op_ps.tile([P, 512], F32)
                for oc in range(OC):
                    nc.tensor.matmul(
                        ps[:],
                        attT[:, oc, s16 * P : (s16 + 1) * P],
                        wo_sb[:, oc, ch * 512 : (ch + 1) * 512],
                        start=(oc == 0),
                        stop=(oc == OC - 1),
                    )
                ob = ob_pool.tile([P, 512], F32)
                nc.vector.tensor_copy(ob[:], ps[:])
                nc.sync.dma_start(
                    outp[s16 * P : (s16 + 1) * P, ch * 512 : (ch + 1) * 512],
                    ob[:],
                )


def build(T=2048):
    nc = bass.Bass()
    with tile.TileContext(nc) as tc:
        with ExitStack() as ctx:
            _emit(nc, tc, ctx, T)
    return nc


def make_stair():
    j = np.arange(P)[:, None]
    u = np.arange(1024)[None, :]
    return (u >= j + 512).astype(ml_dtypes.bfloat16)


def make_in_maps(x, wq, bq, wk, bk, wv, bv, wo):
    bf = ml_dtypes.bfloat16
    stair = make_stair()
    in_maps = []
    for c in range(8):
        b, g = c // 2, c % 2
        sl = slice(g * CL, (g + 1) * CL)
        in_maps.append(
            {
                "xb": np.ascontiguousarray(x[b].T).astype(bf),
                "wq": np.ascontiguousarray(wq[:, sl]).astype(bf),
                "wk": np.ascontiguousarray(wk[:, sl]).astype(bf),
                "wv": np.ascontiguousarray(wv[:, sl]).astype(bf),
                "wo": np.ascontiguousarray(wo[sl, :]).astype(bf),
                "bq": np.ascontiguousarray(bq[sl]),
                "bk": np.ascontiguousarray(bk[sl]),
                "bvb": np.ascontiguousarray(
                    np.broadcast_to(bv[sl][None, :], (P, CL))
                ).astype(bf),
                "stair": stair,
            }
        )
    return in_maps


_cache = {}


def _split_multi_waits(bir_json: bytes) -> bytes:
    """Split instructions carrying >1 sync waits into single-wait NoOp
    chains on the same engine queue.  The TPB instruction encoding has one
    wait slot; this walrus build refuses multi-wait instructions instead
    of splitting them itself."""
    import orjson

    m = orjson.loads(bir_json)
    n = 0
    for fn in m.get("functions", []):
        for blk in fn.get("blocks", []):
            out = []
            for inst in blk.get("instructions", []):
                si = inst.get("sync_info")
                waits = si.get("on_wait") if si else None
                if waits and len(waits) > 1:
                    for w in waits[:-1]:
                        n += 1
                        out.append(
                            {
                                "debug": inst.get("debug", {}),
                                "engine": inst["engine"],
                                "ins": [],
                                "outs": [],
                                "name": f"{inst['name']}_sw{n}",
                                "opcode": "NoOp",
                                "text_hint": "split_wait",
                                "sync_info": {"on_wait": [w], "on_update": []},
                            }
                        )
                    si["on_wait"] = [waits[-1]]
                out.append(inst)
            blk["instructions"] = out
    return orjson.dumps(m)


def _install_compile_patch():
    import subprocess

    import concourse.bass_utils as bu

    if getattr(bu, "_split_waits_patched", False):
        return
    orig = bu.compile_bir_kernel

    def patched(bir_json, tmpdir, neff_name="file.neff"):
        return orig(_split_multi_waits(bir_json), tmpdir, neff_name)

    bu.compile_bir_kernel = patched
    bu._split_waits_patched = True
    try:
        import concourse.bass2jax as b2j

        b2j.compile_bir_kernel = patched
    except ImportError:
        pass

    # note: --enable-ldw-opt=true was tried to hide the per-matmul
    # LDWEIGHTS cost, but this walrus build fails codegen with it
    # (visitInstLdweights error), so it stays at the default false.
    del subprocess


def kernel(x, wq, bq, wk, bk, wv, bv, wo, bo):
    from concourse.bass_utils import run_bass_kernel_spmd

    _install_compile_patch()

    x = np.asarray(x, np.float32)
    args = [np.asarray(a, np.float32) for a in (wq, bq, wk, bk, wv, bv, wo, bo)]
    wq, bq, wk, bk, wv, bv, wo, bo = args
    B, T, _ = x.shape

    if "nc" not in _cache:
        _cache["nc"] = build(T)
    nc = _cache["nc"]

    in_maps = make_in_maps(x, wq, bq, wk, bk, wv, bv, wo)
    res = run_bass_kernel_spmd(nc, in_maps, core_ids=list(range(8)))
    out = np.empty((B, T, C), np.float32)
    for b in range(B):
        out[b] = res.results[2 * b]["outp"] + res.results[2 * b + 1]["outp"] + bo
    return out


# revision 29
# speedup vs baseline: 1.0009x; 1.0009x over previous
"""Causal self-attention Trainium2 kernel.

Sharding: 8 cores = (4 batches) x (2 head-groups of 8 heads).
Each core: projections for its 512 channels, causal attention for its 8
heads over its batch, partial out-projection over its 512 channels.
Host: sums the two partials per batch and adds the output bias.

All matmul operands are bf16 (accumulation in fp32 PSUM); softmax,
normalization and the output partials stay fp32.

Key optimizations vs the naive version:
  - x is transposed on the HOST: xb arrives [C, T] so xT loads are plain
    per-fc direct DMAs (no DMA-transpose; clean 4KB-contiguous rows).
  - weight loads split per-fc and spread over 4 engine DMA queues so the
    first projection matmul starts ~10us in, not 44us.
  - v bias comes in pre-broadcast [128, 512] from the host (replaces 16
    PE ones-trick matmuls with the existing DVE copy turned tensor_add).
  - softmax normalize is software-pipelined: per block, reciprocal runs
    via the fast custom-DVE approx, and the PE row-broadcast matmul for
    block b is issued between score groups of block b+1 so the in-order
    PE queue never stalls on the DVE.
  - score groups are issued one group ahead of AV so the PE streams
    scores of group g+1 while ACT exps group g.
  - AV matmuls and stair masking are restricted to the not-always-masked
    query columns of diagonal tiles (exact: the skipped ex columns are
    never read).

Layouts on core (b = fixed batch, channels o in [0,512) local):
  xT   [128f, 8fc, 2048t] bf16 - direct per-fc DMA from host-transposed x
  qT/kT [128o, 4oc, 2048t] bf16 - head h = oc*2+hh on partitions hh*64..+64
  vx   [128t, 16tj, 8h*65] bf16 - v natural + ones column per head (denom)
  scores^T tiles [128j, 512i] f32 psum -> exp on ACT (scale=1/8) -> bf16
  attn^T accum psum [65, 512i] f32: rows 0..63 head out, row 64 denom
  out  psum [128t, 512c] f32 -> sbuf -> DRAM partial
"""

from contextlib import ExitStack

import ml_dtypes
import numpy as np

import concourse.bass as bass
import concourse.mybir as mybir
import concourse.tile as tile

P = 128
C = 1024  # d_model
CL = 512  # local channels (8 heads * 64)
D = 64  # head dim
NH = 8  # local heads
FC = C // P  # 8 f-chunks
OC = CL // P  # 4 o-chunks
F32 = mybir.dt.float32
BF16 = mybir.dt.bfloat16
AF = mybir.ActivationFunctionType
GROUP = 2  # score jt-tiles per exp call (2 psum banks, triple buffered)
LOOKAHEAD = 2  # score groups in flight ahead of their AV matmuls


def _emit(nc, tc, ctx, T):
    NT = T // P  # 128-token chunks
    T4 = T // 512  # 512-token chunks

    xb = nc.dram_tensor("xb", [C, T], BF16, kind="ExternalInput")
    wq_d = nc.dram_tensor("wq", [C, CL], BF16, kind="ExternalInput")
    wk_d = nc.dram_tensor("wk", [C, CL], BF16, kind="ExternalInput")
    wv_d = nc.dram_tensor("wv", [C, CL], BF16, kind="ExternalInput")
    wo_d = nc.dram_tensor("wo", [CL, C], BF16, kind="ExternalInput")
    bq_d = nc.dram_tensor("bq", [CL], F32, kind="ExternalInput")
    bk_d = nc.dram_tensor("bk", [CL], F32, kind="ExternalInput")
    bvb_d = nc.dram_tensor("bvb", [P, CL], BF16, kind="ExternalInput")
    stair_d = nc.dram_tensor("stair", [P, 1024], BF16, kind="ExternalInput")
    outp = nc.dram_tensor("outp", [T, C], F32, kind="ExternalOutput")

    const = ctx.enter_context(tc.tile_pool(name="const", bufs=1))
    ones1 = const.tile([1, P], BF16)
    bq_sb = const.tile([P, OC], F32)
    bk_sb = const.tile([P, OC], F32)
    bvb_sb = const.tile([P, CL], BF16)
    stair_sb = const.tile([P, 1024], BF16)

    qkv = ctx.enter_context(tc.tile_pool(name="qkv", bufs=1))
    qT = qkv.tile([P, OC, T], BF16)
    kT = qkv.tile([P, OC, T], BF16)
    vx = qkv.tile([P, NT, NH * 65], BF16)
    vx5 = vx.rearrange("p n (h u) -> p n h u", u=65)

    # ---------------- projections ----------------
    with (
        tc.tile_pool(name="wpool", bufs=1) as wpool,
        tc.tile_pool(name="xT_pool", bufs=1) as xT_pool,
        tc.tile_pool(name="pj_ps", bufs=8, space="PSUM") as pj_ps,
    ):
        xT = xT_pool.tile([P, FC, T], BF16)
        wq_sb = wpool.tile([P, FC, CL], BF16)
        wk_sb = wpool.tile([P, FC, CL], BF16)
        wv_sb = wpool.tile([P, FC, CL], BF16)
        xbr = xb.rearrange("(fc p) t -> p fc t", p=P)
        wqr = wq_d.rearrange("(fc p) o -> p fc o", p=P)
        wkr = wk_d.rearrange("(fc p) o -> p fc o", p=P)
        wvr = wv_d.rearrange("(fc p) o -> p fc o", p=P)
        # spread the loads over the three DMA-capable queues; per-fc
        # granularity so the first matmuls start as soon as chunk 0 lands.
        # x chunks alternate between two queues to double early supply
        # (the q+k projections consume each xT chunk within ~3us).
        # Constants / memsets issue AFTER these: they are needed late.
        for fc in range(FC):
            if fc % 2 == 0:
                nc.sync.dma_start(xT[:, fc, :], xbr[:, fc, :])
            else:
                nc.gpsimd.dma_start(xT[:, fc, :], xbr[:, fc, :])
            nc.gpsimd.dma_start(wq_sb[:, fc, :], wqr[:, fc, :])
            nc.scalar.dma_start(wk_sb[:, fc, :], wkr[:, fc, :])
        for fc in range(FC):
            nc.sync.dma_start(wv_sb[:, fc, :], wvr[:, fc, :])
        nc.scalar.dma_start(bq_sb[:], bq_d.rearrange("(oc p) -> p oc", p=P))
        nc.scalar.dma_start(bk_sb[:], bk_d.rearrange("(oc p) -> p oc", p=P))
        nc.scalar.dma_start(bvb_sb[:], bvb_d[:])
        nc.scalar.dma_start(stair_sb[:], stair_d[:])
        nc.gpsimd.memset(ones1[:], 1.0)
        nc.gpsimd.memset(vx5[:, :, :, 64:65], 1.0)

        for oc in range(OC):
            # q and k interleaved per fc chunk: 8 matmuls per arriving
            # xT chunk instead of 4, so the PE keeps up with the DMAs.
            pq = [
                pj_ps.tile([P, 512], F32, tag="pj", name=f"pjq{oc}_{tt}")
                for tt in range(T4)
            ]
            pk = [
                pj_ps.tile([P, 512], F32, tag="pj", name=f"pjk{oc}_{tt}")
                for tt in range(T4)
            ]
            for fc in range(FC):
                for w_sb, pss in ((wq_sb, pq), (wk_sb, pk)):
                    for tt in range(T4):
                        nc.tensor.matmul(
                            pss[tt][:],
                            w_sb[:, fc, oc * P : (oc + 1) * P],
                            xT[:, fc, tt * 512 : (tt + 1) * 512],
                            start=(fc == 0),
                            stop=(fc == FC - 1),
                        )
            for b_sb, dT, pss in ((bq_sb, qT, pq), (bk_sb, kT, pk)):
                for tt in range(T4):
                    nc.vector.tensor_scalar_add(
                        dT[:, oc, tt * 512 : (tt + 1) * 512],
                        pss[tt][:],
                        b_sb[:, oc : oc + 1],
                    )
        for s in range(NT):
            ps = pj_ps.tile([P, 512], F32, tag="pj", name=f"pjv{s}")
            for fc in range(FC):
                nc.tensor.matmul(
                    ps[:],
                    xT[:, fc, s * P : (s + 1) * P],
                    wv_sb[:, fc, :],
                    start=(fc == 0),
                    stop=(fc == FC - 1),
                )
            nc.vector.tensor_add(
                vx5[:, s, :, 0:64],
                ps[:].rearrange("p (h d) -> p h d", d=D),
                bvb_sb[:].rearrange("p (h d) -> p h d", d=D),
            )

    # ---------------- attention ----------------
    wo_pool = ctx.enter_context(tc.tile_pool(name="wo_pool", bufs=1))
    attT_pool = ctx.enter_context(tc.tile_pool(name="attT_pool", bufs=1))
    wo_sb = wo_pool.tile([P, OC, C], BF16)
    nc.gpsimd.dma_start(wo_sb[:], wo_d.rearrange("(oc p) c -> p oc c", p=P))
    attT = attT_pool.tile([P, OC, T], BF16)

    with (
        tc.tile_pool(name="exp_pool", bufs=LOOKAHEAD + 1) as exp_pool,
        tc.tile_pool(name="nrm", bufs=2) as nrm_pool,
        tc.tile_pool(name="sc_ps", bufs=LOOKAHEAD + 1, space="PSUM") as sc_ps_pool,
        tc.tile_pool(name="at_ps", bufs=2, space="PSUM") as at_ps_pool,
    ):
        pending = None  # (at, rc_bf, oc, base, ic) awaiting normalize
        pending_recips = []  # lazy [1,128] reciprocal chunks

        def issue_recips(k):
            for _ in range(min(k, len(pending_recips))):
                r_rc, r_at, q4 = pending_recips.pop(0)
                with nc.allow_low_precision(reason="softmax recip bcast"):
                    nc.vector.reciprocal(
                        r_rc[:, q4 * 128 : (q4 + 1) * 128],
                        r_at[64:65, q4 * 128 : (q4 + 1) * 128],
                    )

        def flush_normalize():
            nonlocal pending
            if pending is None:
                return
            issue_recips(4)
            p_at, p_rc, p_oc, p_base, p_ic = pending
            pending = None
            # broadcast recip row into psum rows 64..127 (K=1 outer)
            nc.tensor.matmul(
                p_at[64:128, :], ones1[:, 0:64], p_rc[:], start=True, stop=True
            )
            tmp = nrm_pool.tile([64, 512], F32, tag="tmp")
            nc.scalar.copy(tmp[:], p_at[0:64, :])
            nc.vector.tensor_mul(
                attT[p_base : p_base + D, p_oc, p_ic * 512 : (p_ic + 1) * 512],
                tmp[:],
                p_at[64:128, :],
            )

        # flat software pipeline over all (head, query-block) blocks: the
        # PE issue order interleaves score groups one ahead of AV groups
        # ACROSS block boundaries, so exp/stair latency is always covered
        # by score streaming, and the normalize broadcast of block b sits
        # ~2 score groups + 1 AV group after b's last AV (covers the
        # 3.4us DVE reciprocal).
        def issue_av(s):
            nonlocal pending
            (s_at, s_h, s_ic, s_njt, grp, ex, is_last, gi) = s
            if gi == 1:
                flush_normalize()
            for si, jt in enumerate(grp):
                d = jt - s_ic * 4
                lo = d * P if d > 0 else 0
                nc.tensor.matmul(
                    s_at[0:65, lo:512],
                    vx5[:, jt, s_h, :],
                    ex[:, si, lo:512],
                    start=(jt == 0),
                    stop=(jt == s_njt - 1),
                )

        def finish_block(s):
            nonlocal pending
            (s_at, s_h, s_ic, s_njt, grp, ex, is_last, gi) = s
            rc = nrm_pool.tile([1, 512], BF16, tag="rc")
            # reciprocal as 4 lazily-issued [1,128] chunks: a single [1,512]
            # reciprocal is 3.4us of in-order DVE queue time that blocks the
            # next block's stair masks (which gate its AV matmuls); chunks
            # issued between stair groups let the DVE interleave them
            for q4 in range(4):
                pending_recips.append((rc, s_at, q4))
            oc, hh = s_h // 2, s_h % 2
            pending = (s_at, rc, oc, hh * 64, s_ic)

        # Block order pairs a small block with a big one (hh0 ascending ic
        # interleaved with hh1 descending, parity flipped per oc) so the
        # ~3.8us of reciprocal chunks of a just-finished small block always
        # has a big block's scores/AVs to hide behind.
        block_seq = []
        for oc in range(OC):
            ics = list(range(T4))
            a, b = (ics, ics[::-1]) if oc % 2 == 0 else (ics[::-1], ics)
            for i in range(T4):
                block_seq.append((oc, 0, a[i]))
                block_seq.append((oc, 1, b[i]))

        staged = []  # score-group lookahead queue (depth LOOKAHEAD)
        if True:
            for oc, hh, ic in block_seq:
                h = oc * 2 + hh
                base = hh * 64
                if True:
                    njt = ic * 4 + 4
                    at = at_ps_pool.tile([P, 512], F32)
                    groups = [
                        list(range(g0, min(g0 + GROUP, njt)))
                        for g0 in range(0, njt, GROUP)
                    ]
                    for gi, grp in enumerate(groups):
                        n = len(grp)
                        sc = sc_ps_pool.tile([P, GROUP, 512], F32)
                        for si, jt in enumerate(grp):
                            nc.tensor.matmul(
                                sc[:, si, :],
                                kT[base : base + D, oc, jt * P : (jt + 1) * P],
                                qT[base : base + D, oc, ic * 512 : (ic + 1) * 512],
                                start=True,
                                stop=True,
                            )
                        ex = exp_pool.tile([P, GROUP, 512], BF16)
                        nc.scalar.activation(
                            ex[:, 0:n, :], sc[:, 0:n, :], AF.Exp, scale=0.125
                        )
                        for si, jt in enumerate(grp):
                            d = jt - ic * 4
                            if d >= 0:
                                # zero the sub-diagonal triangle of the
                                # 128-wide diagonal strip; columns left of
                                # it are skipped by the restricted AV.
                                nc.vector.tensor_mul(
                                    ex[:, si, d * P : (d + 1) * P],
                                    ex[:, si, d * P : (d + 1) * P],
                                    stair_sb[:, 512 : 512 + P],
                                )
                        issue_recips(2)
                        if len(staged) >= LOOKAHEAD:
                            s = staged.pop(0)
                            issue_av(s)
                            if s[6]:
                                finish_block(s)
                        staged.append(
                            (at, h, ic, njt, grp, ex, gi == len(groups) - 1, gi)
                        )
        for s in staged:
            issue_av(s)
            if s[6]:
                finish_block(s)
        flush_normalize()

    # ---------------- out-projection ----------------
    with (
        tc.tile_pool(name="op_ps", bufs=4, space="PSUM") as op_ps,
        tc.tile_pool(name="ob_pool", bufs=4) as ob_pool,
    ):
        for s16 in range(NT):
            for ch in range(2):
                ps = op_ps.tile([P, 512], F32)
                for oc in range(OC):
                    nc.tensor.matmul(
                        ps[:],
                        attT[:, oc, s16 * P : (s16 + 1) * P],
                        wo_sb[:, oc, ch * 512 : (ch + 1) * 512],
                        start=(oc == 0),
                        stop=(oc == OC - 1),
                    )
                ob = ob_pool.tile([P, 512], F32)
                nc.vector.tensor_copy(ob[:], ps[:])
                nc.sync.dma_start(
                    outp[s16 * P : (s16 + 1) * P, ch * 512 : (ch + 1) * 512],
                    ob[:],
                )


def build(T=2048):
    nc = bass.Bass()
    with tile.TileContext(nc) as tc:
        with ExitStack() as ctx:
            _emit(nc, tc, ctx, T)
    return nc


def make_stair():
    j = np.arange(P)[:, None]
    u = np.arange(1024)[None, :]
    return (u >= j + 512).astype(ml_dtypes.bfloat16)


def make_in_maps(x, wq, bq, wk, bk, wv, bv, wo):
    bf = ml_dtypes.bfloat16
    stair = make_stair()
    in_maps = []
    for c in range(8):
        b, g = c // 2, c % 2
        sl = slice(g * CL, (g + 1) * CL)
        in_maps.append(
            {
                "xb": np.ascontiguousarray(x[b].T).astype(bf),
                "wq": np.ascontiguousarray(wq[:, sl]).astype(bf),
                "wk": np.ascontiguousarray(wk[:, sl]).astype(bf),
                "wv": np.ascontiguousarray(wv[:, sl]).astype(bf),
                "wo": np.ascontiguousarray(wo[sl, :]).astype(bf),
                "bq": np.ascontiguousarray(bq[sl]),
                "bk": np.ascontiguousarray(bk[sl]),
                "bvb": np.ascontiguousarray(
                    np.broadcast_to(bv[sl][None, :], (P, CL))
                ).astype(bf),
                "stair": stair,
            }
        )
    return in_maps


_cache = {}


def _split_multi_waits(bir_json: bytes) -> bytes:
    """Split instructions carrying >1 sync waits into single-wait NoOp
    chains on the same engine queue.  The TPB instruction encoding has one
    wait slot; this walrus build refuses multi-wait instructions instead
    of splitting them itself."""
    import orjson

    m = orjson.loads(bir_json)
    n = 0
    for fn in m.get("functions", []):
        for blk in fn.get("blocks", []):
            out = []
            for inst in blk.get("instructions", []):
                si = inst.get("sync_info")
                waits = si.get("on_wait") if si else None
                if waits and len(waits) > 1:
                    for w in waits[:-1]:
                        n += 1
                        out.append(
                            {
                                "debug": inst.get("debug", {}),
                                "engine": inst["engine"],
                                "ins": [],
                                "outs": [],
                                "name": f"{inst['name']}_sw{n}",
                                "opcode": "NoOp",
                                "text_hint": "split_wait",
                                "sync_info": {"on_wait": [w], "on_update": []},
                            }
                        )
                    si["on_wait"] = [waits[-1]]
                out.append(inst)
            blk["instructions"] = out
    return orjson.dumps(m)


def _install_compile_patch():
    import subprocess

    import concourse.bass_utils as bu

    if getattr(bu, "_split_waits_patched", False):
        return
    orig = bu.compile_bir_kernel

    def patched(bir_json, tmpdir, neff_name="file.neff"):
        return orig(_split_multi_waits(bir_json), tmpdir, neff_name)

    bu.compile_bir_kernel = patched
    bu._split_waits_patched = True
    try:
        import concourse.bass2jax as b2j

        b2j.compile_bir_kernel = patched
    except ImportError:
        pass

    # note: --enable-ldw-opt=true was tried to hide the per-matmul
    # LDWEIGHTS cost, but this walrus build fails codegen with it
    # (visitInstLdweights error), so it stays at the default false.
    del subprocess


def kernel(x, wq, bq, wk, bk, wv, bv, wo, bo):
    from concourse.bass_utils import run_bass_kernel_spmd

    _install_compile_patch()

    x = np.asarray(x, np.float32)
    args = [np.asarray(a, np.float32) for a in (wq, bq, wk, bk, wv, bv, wo, bo)]
    wq, bq, wk, bk, wv, bv, wo, bo = args
    B, T, _ = x.shape

    if "nc" not in _cache:
        _cache["nc"] = build(T)
    nc = _cache["nc"]

    in_maps = make_in_maps(x, wq, bq, wk, bk, wv, bv, wo)
    res = run_bass_kernel_spmd(nc, in_maps, core_ids=list(range(8)))
    out = np.empty((B, T, C), np.float32)
    for b in range(B):
        out[b] = res.results[2 * b]["outp"] + res.results[2 * b + 1]["outp"] + bo
    return out
